# revision 32
# baseline (speedup 1.0000x reference)
"""FBPinn (16-window 1D PINN ensemble) forward pass on 8 Trainium2 NeuronCores.

Strategy (MoE-style routing, expert-parallel over windows):
  - Each of the 100k points lies strictly inside only 1-2 of the 16
    overlapping subdomains, so we route: gather the points of each window
    on the host, run each window's small MLP only on its own points, and
    scatter-add the windowed outputs back.
  - Core c owns windows (2c, 2c+1). The two windows are packed into the
    128-partition dimension (64 neurons each) so every matmul runs with
    K=M=128 via block-diagonal weights, and every tanh runs on all 128
    ACT lanes.
  - Layer 1 (per-point affine) is folded into a K=2 outer-product matmul
    with host-folded scales a = W_in/std, c = b_in - W_in*mean/std.
  - Window functions sigmoid((x-l)/s)*sigmoid(-(x-r)/s) are computed on
    device as (1+tanh(x-l))*(1+tanh(r-x))/4 (same table set as the MLP
    tanh), with the 1/4 folded into W_out/b_out on the host.
"""

import numpy as np

# Problem constants (mirrors reference.py static config)
NW = 16
D0, D1 = 0.0, 100.0
OVERLAP = 0.25
SIGMA = 0.5
NEURONS = 64
N = 100_000

NCORES = 8
NPAD = 8192          # per-window padded point count (max real count is 7930)
F = 1024             # chunk width (points) per tanh activation
NCHUNK = NPAD // F
NBLK = NPAD // 512   # 512-wide output blocks per window

_state: dict = {}


def _geometry():
    width = (D1 - D0) / NW
    i = np.arange(NW)
    lo = np.where(i == 0, D0, D0 + (i - OVERLAP / 2) * width)
    hi = np.where(i == NW - 1, D1, D0 + (i + 1 + OVERLAP / 2) * width)
    means = (lo + hi) / 2
    std = (hi - lo) / 2
    ovm = np.empty(NW + 1)
    ovm[0] = lo[0]
    ovm[NW] = hi[-1]
    ovm[1:NW] = (hi[:-1] + lo[1:]) / 2
    f32 = lambda a: np.asarray(a, np.float32)
    return f32(lo), f32(hi), f32(means), f32(std), f32(ovm)


def _build_nc():
    import concourse.bass as bass  # noqa: F401
    import concourse.tile as tile
    from concourse import bacc, mybir

    f32 = mybir.dt.float32
    f32r = mybir.dt.float32r  # 1-pass reduced-precision fp32 matmul
    AF = mybir.ActivationFunctionType
    ALU = mybir.AluOpType

    nc = bacc.Bacc("TRN2", target_bir_lowering=False, debug=False)

    # batched inputs: xin = xpair (normalized), wm = [lh0 | lh1 | lout],
    # cm = [x32 | wb], bv = [b_in | b_h0 | b_h1 | W_in]
    d_xin = nc.dram_tensor("xin", [2, NPAD], f32, kind="ExternalInput")
    d_wm = nc.dram_tensor("wm", [128, 258], f32r, kind="ExternalInput")
    d_bv = nc.dram_tensor("bv", [128, 4], f32, kind="ExternalInput")
    d_cm = nc.dram_tensor("cm", [32, 515], f32, kind="ExternalInput")
    d_out = nc.dram_tensor("out", [32, 512], f32, kind="ExternalOutput")

    with tile.TileContext(nc) as tc:
        with (
            tc.tile_pool(name="consts", bufs=1) as cp,
            tc.tile_pool(name="hp", bufs=4) as hp,
            tc.tile_pool(name="xbp", bufs=3) as xbp,
            tc.tile_pool(name="pp", bufs=3, space="PSUM") as pp,
            tc.tile_pool(name="pop", bufs=2, space="PSUM") as pop,
            tc.tile_pool(name="mp", bufs=1) as mp,
        ):
            cm = cp.tile([32, 515], f32, tag="cm")
            nc.sync.dma_start(cm[:], d_cm[:])
            wm = cp.tile([128, 258], f32r, tag="wm")
            nc.scalar.dma_start(wm[:], d_wm[:])
            # one tile per window row: partition_broadcast reads partition 0
            # of its source tile
            xinA = cp.tile([1, NPAD], f32, tag="xinA")
            nc.gpsimd.dma_start(xinA[:], d_xin[0:1, :])
            xinB = cp.tile([1, NPAD], f32, tag="xinB")
            nc.gpsimd.dma_start(xinB[:], d_xin[1:2, :])
            bv = cp.tile([128, 4], f32, tag="bv")
            nc.sync.dma_start(bv[:], d_bv[:])

            lh0 = wm[:, 0:128]
            lh1 = wm[:, 128:256]
            lout = wm[:, 256:258]
            x32 = cm[:, 0:512]
            wb = cm[:, 512:515]

            oacc2 = mp.tile([2, NPAD], f32, tag="oacc2")
            oacc = mp.tile([32, 512], f32, tag="oacc")
            h3big = mp.tile([128, NPAD], f32r, tag="h3big")

            # Window weights early (ACT fills while PE warms up):
            # 4*win = (1+tanh(x-ovm_w)) * (1+tanh(ovm_{w+1}-x))
            wtL = mp.tile([32, 512], f32, tag="wtL")
            nc.scalar.activation(wtL[:], x32, AF.Tanh, bias=wb[:, 0:1], scale=1.0)
            wtR = mp.tile([32, 512], f32, tag="wtR")
            nc.scalar.activation(wtR[:], x32, AF.Tanh, bias=wb[:, 1:2], scale=-1.0)

            def emit_out(j):
                for s in range(F // 512):
                    n = j * (F // 512) + s
                    pout = pop.tile([2, 512], f32, tag="po", name=f"po_{n}")
                    nc.tensor.matmul(
                        pout[:], lout, h3big[:, n * 512 : (n + 1) * 512],
                        start=True, stop=True,
                    )
                    nc.vector.tensor_copy(oacc2[:, n * 512 : (n + 1) * 512], pout[:])
                    # reshuffle into [32, 512] combine layout as soon as ready
                    nc.sync.dma_start(
                        oacc[2 * n : 2 * n + 2, :], oacc2[:, n * 512 : (n + 1) * 512]
                    )

            # ---- skewed software pipeline (wavefront over chunks) ----
            h1s, h2s = {}, {}

            def stage1(j):
                # broadcast xn across the 64 neuron partitions of each window,
                # then layer 1 entirely on ACT: h1 = tanh(W_in*xn + b_in)
                xsl = slice(j * F, (j + 1) * F)
                xb = xbp.tile([128, F], f32, tag="xb", name=f"xb_{j}")
                nc.gpsimd.partition_broadcast(xb[0:64, :], xinA[:, xsl])
                # partition_broadcast can only write at base partition 0, so
                # stage window B at base 0 and DVE-copy up (64 is 32-aligned)
                xbB = xbp.tile([64, F], f32, tag="xbB", name=f"xbB_{j}")
                nc.gpsimd.partition_broadcast(xbB[:, :], xinB[:, xsl])
                nc.vector.tensor_copy(xb[64:128, :], xbB[:])
                h1 = hp.tile([128, F], f32r, tag="h1", name=f"h1_{j}")
                nc.scalar.activation(
                    h1[:], xb[:], AF.Tanh, bias=bv[:, 0:1], scale=bv[:, 3:4]
                )
                h1s[j] = h1

            def stage2(j):
                p2 = pp.tile([128, F], f32, tag="ps", name=f"p2_{j}")
                for s in range(F // 512):
                    sl = slice(s * 512, (s + 1) * 512)
                    nc.tensor.matmul(p2[:, sl], lh0, h1s[j][:, sl], start=True, stop=True)
                h2 = hp.tile([128, F], f32r, tag="h2", name=f"h2_{j}")
                nc.scalar.activation(h2[:], p2[:], AF.Tanh, bias=bv[:, 1:2])
                h2s[j] = h2

            def stage3(j):
                p3 = pp.tile([128, F], f32, tag="ps", name=f"p3_{j}")
                for s in range(F // 512):
                    sl = slice(s * 512, (s + 1) * 512)
                    nc.tensor.matmul(p3[:, sl], lh1, h2s[j][:, sl], start=True, stop=True)
                nc.scalar.activation(
                    h3big[:, j * F : (j + 1) * F], p3[:], AF.Tanh, bias=bv[:, 2:3]
                )
                emit_out(j)

            for t in range(NCHUNK + 2):
                if t < NCHUNK:
                    stage1(t)
                if 1 <= t < NCHUNK + 1:
                    stage2(t - 1)
                if t >= 2:
                    stage3(t - 2)

            tp = mp.tile([32, 512], f32, tag="tp")
            nc.vector.tensor_scalar_add(tp[:], wtL[:], 1.0)
            win4 = mp.tile([32, 512], f32, tag="win4")
            nc.vector.scalar_tensor_tensor(
                win4[:], wtR[:], 1.0, tp[:], op0=ALU.add, op1=ALU.mult
            )
            fin = mp.tile([32, 512], f32, tag="fin")
            nc.vector.scalar_tensor_tensor(
                fin[:], oacc[:], wb[:, 2:3], win4[:], op0=ALU.add, op1=ALU.mult
            )
            nc.sync.dma_start(d_out[:], fin[:])

    nc.compile()
    return nc


def _get_nc():
    if "nc" not in _state:
        _state["nc"] = _build_nc()
    return _state["nc"]


def _prepare(x, W_in, b_in, W_h, b_h, W_out, b_out):
    x = np.asarray(x, np.float32)
    W_in = np.asarray(W_in, np.float32)
    b_in = np.asarray(b_in, np.float32)
    W_h = np.asarray(W_h, np.float32)
    b_h = np.asarray(b_h, np.float32)
    W_out = np.asarray(W_out, np.float32)
    b_out = np.asarray(b_out, np.float32)

    lo, hi, means, std, ovm = _geometry()

    # ---- host routing: gather each window's points ----
    idxs, counts = [], []
    for w in range(NW):
        idx = np.nonzero((lo[w] < x) & (x < hi[w]))[0]
        assert len(idx) <= NPAD, f"window {w} has {len(idx)} points > NPAD={NPAD}"
        idxs.append(idx)
        counts.append(len(idx))

    in_maps = []
    for c in range(NCORES):
        A, B = 2 * c, 2 * c + 1
        xA = np.full(NPAD, means[A], np.float32)
        xA[: counts[A]] = x[idxs[A]]
        xB = np.full(NPAD, means[B], np.float32)
        xB[: counts[B]] = x[idxs[B]]
        # normalized per-window inputs (matches reference's xn exactly)
        xin = np.stack([(xA - means[A]) / std[A], (xB - means[B]) / std[B]])

        bv = np.empty((128, 4), np.float32)
        bv[:64, 0] = b_in[A]
        bv[64:, 0] = b_in[B]
        bv[:64, 1] = b_h[0, A]
        bv[64:, 1] = b_h[0, B]
        bv[:64, 2] = b_h[1, A]
        bv[64:, 2] = b_h[1, B]
        bv[:64, 3] = W_in[A]
        bv[64:, 3] = W_in[B]

        # wm = [lh0 | lh1 | lout]
        wm = np.zeros((128, 258), np.float32)
        wm[:64, 0:64] = W_h[0, A]
        wm[64:, 64:128] = W_h[0, B]
        wm[:64, 128:192] = W_h[1, A]
        wm[64:, 192:256] = W_h[1, B]
        wm[:64, 256] = W_out[A] * 0.25
        wm[64:, 257] = W_out[B] * 0.25

        # cm = [x32 | wb]: row 2n = window-A 512-block n, row 2n+1 = B block n
        # wb: col0 = left-edge tanh bias (-ovm_w), col1 = right-edge tanh
        # bias (+ovm_{w+1}), col2 = b_out/4
        cm = np.empty((32, 515), np.float32)
        cm[0::2, :512] = xA.reshape(NBLK, 512)
        cm[1::2, :512] = xB.reshape(NBLK, 512)
        cm[0::2, 512] = -ovm[A]
        cm[1::2, 512] = -ovm[B]
        cm[0::2, 513] = ovm[A + 1]
        cm[1::2, 513] = ovm[B + 1]
        cm[0::2, 514] = b_out[A] * 0.25
        cm[1::2, 514] = b_out[B] * 0.25

        in_maps.append({"xin": xin, "wm": wm, "bv": bv, "cm": cm})

    return in_maps, idxs, counts


def _postprocess(results, idxs, counts):
    pred = np.zeros(N, np.float32)
    for w in range(NW):
        c, s = divmod(w, 2)
        vals = results[c]["out"][s::2].reshape(NPAD)[: counts[w]]
        pred[idxs[w]] += vals
    return pred


def kernel(x, W_in, b_in, W_h, b_h, W_out, b_out):
    from concourse.bass_utils import run_bass_kernel_spmd

    in_maps, idxs, counts = _prepare(x, W_in, b_in, W_h, b_h, W_out, b_out)
    nc = _get_nc()
    res = run_bass_kernel_spmd(nc, in_maps, core_ids=list(range(NCORES)))
    return _postprocess(res.results, idxs, counts)


# revision 35
# speedup vs baseline: 1.2068x; 1.2068x over previous
"""FBPinn (16-window 1D PINN ensemble) forward pass on 8 Trainium2 NeuronCores.

Strategy (MoE-style routing, expert-parallel over windows):
  - Each of the 100k points lies strictly inside only 1-2 of the 16
    overlapping subdomains, so we route: gather the points of each window
    on the host, run each window's small MLP only on its own points, and
    scatter-add the windowed outputs back.
  - Core c owns windows (2c, 2c+1). The two windows are packed into the
    128-partition dimension (64 neurons each) so every matmul runs with
    K=M=128 via block-diagonal weights, and every tanh runs on all 128
    ACT lanes.
  - Layer 1 (per-point affine) is folded into a K=2 outer-product matmul
    with host-folded scales a = W_in/std, c = b_in - W_in*mean/std.
  - Window functions sigmoid((x-l)/s)*sigmoid(-(x-r)/s) are computed on
    device as (1+tanh(x-l))*(1+tanh(r-x))/4 (same table set as the MLP
    tanh), with the 1/4 folded into W_out/b_out on the host.
"""

import numpy as np

# Problem constants (mirrors reference.py static config)
NW = 16
D0, D1 = 0.0, 100.0
OVERLAP = 0.25
SIGMA = 0.5
NEURONS = 64
N = 100_000

NCORES = 8
NPAD = 8192          # per-window padded point count (max real count is 7930)
F = 1024             # chunk width (points) per tanh activation
NCHUNK = NPAD // F
NBLK = NPAD // 512   # 512-wide output blocks per window

_state: dict = {}


def _geometry():
    width = (D1 - D0) / NW
    i = np.arange(NW)
    lo = np.where(i == 0, D0, D0 + (i - OVERLAP / 2) * width)
    hi = np.where(i == NW - 1, D1, D0 + (i + 1 + OVERLAP / 2) * width)
    means = (lo + hi) / 2
    std = (hi - lo) / 2
    ovm = np.empty(NW + 1)
    ovm[0] = lo[0]
    ovm[NW] = hi[-1]
    ovm[1:NW] = (hi[:-1] + lo[1:]) / 2
    f32 = lambda a: np.asarray(a, np.float32)
    return f32(lo), f32(hi), f32(means), f32(std), f32(ovm)


def _build_nc():
    import concourse.bass as bass  # noqa: F401
    import concourse.tile as tile
    from concourse import bacc, mybir

    f32 = mybir.dt.float32
    f32r = mybir.dt.float32r  # 1-pass reduced-precision fp32 matmul
    AF = mybir.ActivationFunctionType
    ALU = mybir.AluOpType

    nc = bacc.Bacc("TRN2", target_bir_lowering=False, debug=False)

    # batched inputs: xin = xpair (normalized), wm = [lh0 | lh1 | lout],
    # cm = [x32 | wb], bv = [b_in | b_h0 | b_h1 | W_in]
    d_xin = nc.dram_tensor("xin", [2, NPAD], f32r, kind="ExternalInput")
    d_lin = nc.dram_tensor("lin", [2, 128], f32r, kind="ExternalInput")
    d_wm = nc.dram_tensor("wm", [128, 258], f32r, kind="ExternalInput")
    d_bv = nc.dram_tensor("bv", [128, 4], f32, kind="ExternalInput")
    d_cm = nc.dram_tensor("cm", [32, 515], f32, kind="ExternalInput")
    d_out = nc.dram_tensor("out", [32, 512], f32, kind="ExternalOutput")

    with tile.TileContext(nc) as tc:
        with (
            tc.tile_pool(name="consts", bufs=1) as cp,
            tc.tile_pool(name="hp", bufs=4) as hp,
            tc.tile_pool(name="pp", bufs=3, space="PSUM") as pp,
            tc.tile_pool(name="pop", bufs=2, space="PSUM") as pop,
            tc.tile_pool(name="mp", bufs=1) as mp,
        ):
            cm = cp.tile([32, 515], f32, tag="cm")
            nc.sync.dma_start(cm[:], d_cm[:])
            wm = cp.tile([128, 258], f32r, tag="wm")
            nc.scalar.dma_start(wm[:], d_wm[:])
            xpr = cp.tile([2, NPAD + 128], f32r, tag="xpr")
            nc.gpsimd.dma_start(xpr[:, 0:NPAD], d_xin[:])
            nc.gpsimd.dma_start(xpr[:, NPAD:], d_lin[:])
            lin = xpr[:, NPAD:]
            bv = cp.tile([128, 4], f32, tag="bv")
            nc.sync.dma_start(bv[:], d_bv[:])

            lh0 = wm[:, 0:128]
            lh1 = wm[:, 128:256]
            lout = wm[:, 256:258]
            x32 = cm[:, 0:512]
            wb = cm[:, 512:515]

            oacc2 = mp.tile([2, NPAD], f32, tag="oacc2")
            oacc = mp.tile([32, 512], f32, tag="oacc")
            h3big = mp.tile([128, NPAD], f32r, tag="h3big")

            # Window weights early (ACT fills while PE warms up):
            # 4*win = (1+tanh(x-ovm_w)) * (1+tanh(ovm_{w+1}-x))
            wtL = mp.tile([32, 512], f32, tag="wtL")
            nc.scalar.activation(wtL[:], x32, AF.Tanh, bias=wb[:, 0:1], scale=1.0)
            wtR = mp.tile([32, 512], f32, tag="wtR")
            nc.scalar.activation(wtR[:], x32, AF.Tanh, bias=wb[:, 1:2], scale=-1.0)

            def emit_out(j):
                for s in range(F // 512):
                    n = j * (F // 512) + s
                    pout = pop.tile([2, 512], f32, tag="po", name=f"po_{n}")
                    nc.tensor.matmul(
                        pout[:], lout, h3big[:, n * 512 : (n + 1) * 512],
                        start=True, stop=True,
                    )
                    nc.vector.tensor_copy(oacc2[:, n * 512 : (n + 1) * 512], pout[:])
                    # reshuffle into [32, 512] combine layout as soon as ready
                    nc.sync.dma_start(
                        oacc[2 * n : 2 * n + 2, :], oacc2[:, n * 512 : (n + 1) * 512]
                    )

            # ---- skewed software pipeline (wavefront over chunks) ----
            h1s, h2s = {}, {}

            def stage1(j):
                xsl = slice(j * F, (j + 1) * F)
                p1 = pp.tile([128, F], f32, tag="ps", name=f"p1_{j}")
                for s in range(F // 512):
                    sl = slice(s * 512, (s + 1) * 512)
                    xs = slice(j * F + s * 512, j * F + (s + 1) * 512)
                    nc.tensor.matmul(p1[:, sl], lin, xpr[:, xs], start=True, stop=True)
                h1 = hp.tile([128, F], f32r, tag="h1", name=f"h1_{j}")
                nc.scalar.activation(h1[:], p1[:], AF.Tanh, bias=bv[:, 0:1])
                h1s[j] = h1

            def stage2(j):
                p2 = pp.tile([128, F], f32, tag="ps", name=f"p2_{j}")
                for s in range(F // 512):
                    sl = slice(s * 512, (s + 1) * 512)
                    nc.tensor.matmul(p2[:, sl], lh0, h1s[j][:, sl], start=True, stop=True)
                h2 = hp.tile([128, F], f32r, tag="h2", name=f"h2_{j}")
                nc.scalar.activation(h2[:], p2[:], AF.Tanh, bias=bv[:, 1:2])
                h2s[j] = h2

            def stage3(j):
                p3 = pp.tile([128, F], f32, tag="ps", name=f"p3_{j}")
                for s in range(F // 512):
                    sl = slice(s * 512, (s + 1) * 512)
                    nc.tensor.matmul(p3[:, sl], lh1, h2s[j][:, sl], start=True, stop=True)
                nc.scalar.activation(
                    h3big[:, j * F : (j + 1) * F], p3[:], AF.Tanh, bias=bv[:, 2:3]
                )
                emit_out(j)

            for t in range(NCHUNK + 2):
                if t < NCHUNK:
                    stage1(t)
                if 1 <= t < NCHUNK + 1:
                    stage2(t - 1)
                if t >= 2:
                    stage3(t - 2)

            tp = mp.tile([32, 512], f32, tag="tp")
            nc.vector.tensor_scalar_add(tp[:], wtL[:], 1.0)
            win4 = mp.tile([32, 512], f32, tag="win4")
            nc.vector.scalar_tensor_tensor(
                win4[:], wtR[:], 1.0, tp[:], op0=ALU.add, op1=ALU.mult
            )
            fin = mp.tile([32, 512], f32, tag="fin")
            nc.vector.scalar_tensor_tensor(
                fin[:], oacc[:], wb[:, 2:3], win4[:], op0=ALU.add, op1=ALU.mult
            )
            nc.sync.dma_start(d_out[:], fin[:])

    nc.compile()
    return nc


def _get_nc():
    if "nc" not in _state:
        _state["nc"] = _build_nc()
    return _state["nc"]


def _prepare(x, W_in, b_in, W_h, b_h, W_out, b_out):
    x = np.asarray(x, np.float32)
    W_in = np.asarray(W_in, np.float32)
    b_in = np.asarray(b_in, np.float32)
    W_h = np.asarray(W_h, np.float32)
    b_h = np.asarray(b_h, np.float32)
    W_out = np.asarray(W_out, np.float32)
    b_out = np.asarray(b_out, np.float32)

    lo, hi, means, std, ovm = _geometry()

    # ---- host routing: gather each window's points ----
    idxs, counts = [], []
    for w in range(NW):
        idx = np.nonzero((lo[w] < x) & (x < hi[w]))[0]
        assert len(idx) <= NPAD, f"window {w} has {len(idx)} points > NPAD={NPAD}"
        idxs.append(idx)
        counts.append(len(idx))

    in_maps = []
    for c in range(NCORES):
        A, B = 2 * c, 2 * c + 1
        xA = np.full(NPAD, means[A], np.float32)
        xA[: counts[A]] = x[idxs[A]]
        xB = np.full(NPAD, means[B], np.float32)
        xB[: counts[B]] = x[idxs[B]]
        # normalized per-window inputs (matches reference's xn exactly)
        xin = np.stack([(xA - means[A]) / std[A], (xB - means[B]) / std[B]])

        lin = np.zeros((2, 128), np.float32)
        lin[0, :64] = W_in[A]
        lin[1, 64:] = W_in[B]

        bv = np.empty((128, 4), np.float32)
        bv[:64, 0] = b_in[A]
        bv[64:, 0] = b_in[B]
        bv[:64, 1] = b_h[0, A]
        bv[64:, 1] = b_h[0, B]
        bv[:64, 2] = b_h[1, A]
        bv[64:, 2] = b_h[1, B]
        bv[:64, 3] = W_in[A]
        bv[64:, 3] = W_in[B]

        # wm = [lh0 | lh1 | lout]
        wm = np.zeros((128, 258), np.float32)
        wm[:64, 0:64] = W_h[0, A]
        wm[64:, 64:128] = W_h[0, B]
        wm[:64, 128:192] = W_h[1, A]
        wm[64:, 192:256] = W_h[1, B]
        wm[:64, 256] = W_out[A] * 0.25
        wm[64:, 257] = W_out[B] * 0.25

        # cm = [x32 | wb]: row 2n = window-A 512-block n, row 2n+1 = B block n
        # wb: col0 = left-edge tanh bias (-ovm_w), col1 = right-edge tanh
        # bias (+ovm_{w+1}), col2 = b_out/4
        cm = np.empty((32, 515), np.float32)
        cm[0::2, :512] = xA.reshape(NBLK, 512)
        cm[1::2, :512] = xB.reshape(NBLK, 512)
        cm[0::2, 512] = -ovm[A]
        cm[1::2, 512] = -ovm[B]
        cm[0::2, 513] = ovm[A + 1]
        cm[1::2, 513] = ovm[B + 1]
        cm[0::2, 514] = b_out[A] * 0.25
        cm[1::2, 514] = b_out[B] * 0.25

        in_maps.append({"xin": xin, "lin": lin, "wm": wm, "bv": bv, "cm": cm})

    return in_maps, idxs, counts


def _postprocess(results, idxs, counts):
    pred = np.zeros(N, np.float32)
    for w in range(NW):
        c, s = divmod(w, 2)
        vals = results[c]["out"][s::2].reshape(NPAD)[: counts[w]]
        pred[idxs[w]] += vals
    return pred


def kernel(x, W_in, b_in, W_h, b_h, W_out, b_out):
    from concourse.bass_utils import run_bass_kernel_spmd

    in_maps, idxs, counts = _prepare(x, W_in, b_in, W_h, b_h, W_out, b_out)
    nc = _get_nc()
    res = run_bass_kernel_spmd(nc, in_maps, core_ids=list(range(NCORES)))
    return _postprocess(res.results, idxs, counts)


# revision 39
# speedup vs baseline: 1.2212x; 1.0120x over previous
"""FBPinn (16-window 1D PINN ensemble) forward pass on 8 Trainium2 NeuronCores.

Strategy (MoE-style routing, expert-parallel over windows):
  - Each of the 100k points lies strictly inside only 1-2 of the 16
    overlapping subdomains, so we route: gather the points of each window
    on the host, run each window's small MLP only on its own points, and
    scatter-add the windowed outputs back.
  - Core c owns windows (2c, 2c+1). The two windows are packed into the
    128-partition dimension (64 neurons each) so every matmul runs with
    K=M=128 via block-diagonal weights, and every tanh runs on all 128
    ACT lanes.
  - Layer 1 (per-point affine) is folded into a K=2 outer-product matmul
    with host-folded scales a = W_in/std, c = b_in - W_in*mean/std.
  - Window functions sigmoid((x-l)/s)*sigmoid(-(x-r)/s) are computed on
    device as (1+tanh(x-l))*(1+tanh(r-x))/4 (same table set as the MLP
    tanh), with the 1/4 folded into W_out/b_out on the host.
"""

import numpy as np

# Problem constants (mirrors reference.py static config)
NW = 16
D0, D1 = 0.0, 100.0
OVERLAP = 0.25
SIGMA = 0.5
NEURONS = 64
N = 100_000

NCORES = 8
NPAD = 8192          # per-window padded point count (max real count is 7930)
F = 1024             # chunk width (points) per tanh activation
NCHUNK = NPAD // F
NBLK = NPAD // 512   # 512-wide output blocks per window

_state: dict = {}


def _geometry():
    width = (D1 - D0) / NW
    i = np.arange(NW)
    lo = np.where(i == 0, D0, D0 + (i - OVERLAP / 2) * width)
    hi = np.where(i == NW - 1, D1, D0 + (i + 1 + OVERLAP / 2) * width)
    means = (lo + hi) / 2
    std = (hi - lo) / 2
    ovm = np.empty(NW + 1)
    ovm[0] = lo[0]
    ovm[NW] = hi[-1]
    ovm[1:NW] = (hi[:-1] + lo[1:]) / 2
    f32 = lambda a: np.asarray(a, np.float32)
    return f32(lo), f32(hi), f32(means), f32(std), f32(ovm)


def _build_nc():
    import concourse.bass as bass  # noqa: F401
    import concourse.tile as tile
    from concourse import bacc, mybir

    f32 = mybir.dt.float32
    f32r = mybir.dt.float32r  # 1-pass reduced-precision fp32 matmul
    AF = mybir.ActivationFunctionType
    ALU = mybir.AluOpType

    nc = bacc.Bacc("TRN2", target_bir_lowering=False, debug=False)

    # batched inputs: xin = xpair (normalized), wm = [lh0 | lh1 | lout],
    # cm = [x32 | wb], bv = [b_in | b_h0 | b_h1 | W_in]
    d_xin = nc.dram_tensor("xin", [2, NPAD], f32r, kind="ExternalInput")
    d_lin = nc.dram_tensor("lin", [2, 128], f32r, kind="ExternalInput")
    d_wm = nc.dram_tensor("wm", [128, 258], f32r, kind="ExternalInput")
    d_bv = nc.dram_tensor("bv", [128, 4], f32, kind="ExternalInput")
    d_cm = nc.dram_tensor("cm", [32, 515], f32, kind="ExternalInput")
    d_out = nc.dram_tensor("out", [32, 512], f32, kind="ExternalOutput")

    with tile.TileContext(nc) as tc:
        with (
            tc.tile_pool(name="consts", bufs=1) as cp,
            tc.tile_pool(name="hp", bufs=4) as hp,
            tc.tile_pool(name="pp", bufs=3, space="PSUM") as pp,
            tc.tile_pool(name="pop", bufs=2, space="PSUM") as pop,
            tc.tile_pool(name="mp", bufs=1) as mp,
        ):
            cm = cp.tile([32, 515], f32, tag="cm")
            nc.sync.dma_start(cm[:], d_cm[:])
            wm = cp.tile([128, 258], f32r, tag="wm")
            nc.scalar.dma_start(wm[:], d_wm[:])
            xpr = cp.tile([2, NPAD + 128], f32r, tag="xpr")
            nc.gpsimd.dma_start(xpr[:, 0:NPAD], d_xin[:])
            nc.gpsimd.dma_start(xpr[:, NPAD:], d_lin[:])
            lin = xpr[:, NPAD:]
            bv = cp.tile([128, 4], f32, tag="bv")
            nc.sync.dma_start(bv[:], d_bv[:])

            lh0 = wm[:, 0:128]
            lh1 = wm[:, 128:256]
            lout = wm[:, 256:258]
            x32 = cm[:, 0:512]
            wb = cm[:, 512:515]

            oacc2 = mp.tile([2, NPAD], f32, tag="oacc2")
            oacc = mp.tile([32, 512], f32, tag="oacc")
            h3big = mp.tile([128, NPAD], f32r, tag="h3big")

            # PE warm-up burst: dummy matmuls during the DMA-load head keep
            # the PE HAM activity window busy so real matmuls start at 2.4GHz
            # instead of the throttled 1.2GHz. Reuses the po psum slots.
            dum = mp.tile([2, 514], mybir.dt.bfloat16, tag="dum")
            nc.vector.memset(dum[:], 0.0)
            for w in range(10):
                pw = pop.tile([2, 512], f32, tag="po", name=f"warm_{w}")
                nc.tensor.matmul(pw[:], dum[:, 512:514], dum[:, 0:512],
                                 start=True, stop=True)

            # Window weights early (ACT fills while PE warms up):
            # 4*win = (1+tanh(x-ovm_w)) * (1+tanh(ovm_{w+1}-x))
            wtL = mp.tile([32, 512], f32, tag="wtL")
            nc.scalar.activation(wtL[:], x32, AF.Tanh, bias=wb[:, 0:1], scale=1.0)
            wtR = mp.tile([32, 512], f32, tag="wtR")
            nc.scalar.activation(wtR[:], x32, AF.Tanh, bias=wb[:, 1:2], scale=-1.0)
            # win4 precomputed up front; only `fin` remains in the tail
            tp = mp.tile([32, 512], f32, tag="tp")
            nc.vector.tensor_scalar_add(tp[:], wtL[:], 1.0)
            win4 = mp.tile([32, 512], f32, tag="win4")
            nc.vector.scalar_tensor_tensor(
                win4[:], wtR[:], 1.0, tp[:], op0=ALU.add, op1=ALU.mult
            )

            def emit_out(j):
                for s in range(F // 512):
                    n = j * (F // 512) + s
                    pout = pop.tile([2, 512], f32, tag="po", name=f"po_{n}")
                    nc.tensor.matmul(
                        pout[:], lout, h3big[:, n * 512 : (n + 1) * 512],
                        start=True, stop=True,
                    )
                    nc.vector.tensor_copy(oacc2[:, n * 512 : (n + 1) * 512], pout[:])
                    # reshuffle into [32, 512] combine layout as soon as ready
                    nc.sync.dma_start(
                        oacc[2 * n : 2 * n + 2, :], oacc2[:, n * 512 : (n + 1) * 512]
                    )

            # ---- skewed software pipeline (wavefront over chunks) ----
            h1s, h2s = {}, {}

            def stage1(j):
                xsl = slice(j * F, (j + 1) * F)
                p1 = pp.tile([128, F], f32, tag="ps", name=f"p1_{j}")
                for s in range(F // 512):
                    sl = slice(s * 512, (s + 1) * 512)
                    xs = slice(j * F + s * 512, j * F + (s + 1) * 512)
                    nc.tensor.matmul(p1[:, sl], lin, xpr[:, xs], start=True, stop=True)
                h1 = hp.tile([128, F], f32r, tag="h1", name=f"h1_{j}")
                nc.scalar.activation(h1[:], p1[:], AF.Tanh, bias=bv[:, 0:1])
                h1s[j] = h1

            def stage2(j):
                p2 = pp.tile([128, F], f32, tag="ps", name=f"p2_{j}")
                for s in range(F // 512):
                    sl = slice(s * 512, (s + 1) * 512)
                    nc.tensor.matmul(p2[:, sl], lh0, h1s[j][:, sl], start=True, stop=True)
                h2 = hp.tile([128, F], f32r, tag="h2", name=f"h2_{j}")
                nc.scalar.activation(h2[:], p2[:], AF.Tanh, bias=bv[:, 1:2])
                h2s[j] = h2

            def stage3(j):
                p3 = pp.tile([128, F], f32, tag="ps", name=f"p3_{j}")
                for s in range(F // 512):
                    sl = slice(s * 512, (s + 1) * 512)
                    nc.tensor.matmul(p3[:, sl], lh1, h2s[j][:, sl], start=True, stop=True)
                nc.scalar.activation(
                    h3big[:, j * F : (j + 1) * F], p3[:], AF.Tanh, bias=bv[:, 2:3]
                )
                emit_out(j)

            for t in range(NCHUNK + 2):
                if t < NCHUNK:
                    stage1(t)
                if 1 <= t < NCHUNK + 1:
                    stage2(t - 1)
                if t >= 2:
                    stage3(t - 2)

            fin = mp.tile([32, 512], f32, tag="fin")
            nc.vector.scalar_tensor_tensor(
                fin[:], oacc[:], wb[:, 2:3], win4[:], op0=ALU.add, op1=ALU.mult
            )
            nc.sync.dma_start(d_out[:], fin[:])

    nc.compile()
    return nc


def _get_nc():
    if "nc" not in _state:
        _state["nc"] = _build_nc()
    return _state["nc"]


def _prepare(x, W_in, b_in, W_h, b_h, W_out, b_out):
    x = np.asarray(x, np.float32)
    W_in = np.asarray(W_in, np.float32)
    b_in = np.asarray(b_in, np.float32)
    W_h = np.asarray(W_h, np.float32)
    b_h = np.asarray(b_h, np.float32)
    W_out = np.asarray(W_out, np.float32)
    b_out = np.asarray(b_out, np.float32)

    lo, hi, means, std, ovm = _geometry()

    # ---- host routing: gather each window's points ----
    idxs, counts = [], []
    for w in range(NW):
        idx = np.nonzero((lo[w] < x) & (x < hi[w]))[0]
        assert len(idx) <= NPAD, f"window {w} has {len(idx)} points > NPAD={NPAD}"
        idxs.append(idx)
        counts.append(len(idx))

    in_maps = []
    for c in range(NCORES):
        A, B = 2 * c, 2 * c + 1
        xA = np.full(NPAD, means[A], np.float32)
        xA[: counts[A]] = x[idxs[A]]
        xB = np.full(NPAD, means[B], np.float32)
        xB[: counts[B]] = x[idxs[B]]
        # normalized per-window inputs (matches reference's xn exactly)
        xin = np.stack([(xA - means[A]) / std[A], (xB - means[B]) / std[B]])

        lin = np.zeros((2, 128), np.float32)
        lin[0, :64] = W_in[A]
        lin[1, 64:] = W_in[B]

        bv = np.empty((128, 4), np.float32)
        bv[:64, 0] = b_in[A]
        bv[64:, 0] = b_in[B]
        bv[:64, 1] = b_h[0, A]
        bv[64:, 1] = b_h[0, B]
        bv[:64, 2] = b_h[1, A]
        bv[64:, 2] = b_h[1, B]
        bv[:64, 3] = W_in[A]
        bv[64:, 3] = W_in[B]

        # wm = [lh0 | lh1 | lout]
        wm = np.zeros((128, 258), np.float32)
        wm[:64, 0:64] = W_h[0, A]
        wm[64:, 64:128] = W_h[0, B]
        wm[:64, 128:192] = W_h[1, A]
        wm[64:, 192:256] = W_h[1, B]
        wm[:64, 256] = W_out[A] * 0.25
        wm[64:, 257] = W_out[B] * 0.25

        # cm = [x32 | wb]: row 2n = window-A 512-block n, row 2n+1 = B block n
        # wb: col0 = left-edge tanh bias (-ovm_w), col1 = right-edge tanh
        # bias (+ovm_{w+1}), col2 = b_out/4
        cm = np.empty((32, 515), np.float32)
        cm[0::2, :512] = xA.reshape(NBLK, 512)
        cm[1::2, :512] = xB.reshape(NBLK, 512)
        cm[0::2, 512] = -ovm[A]
        cm[1::2, 512] = -ovm[B]
        cm[0::2, 513] = ovm[A + 1]
        cm[1::2, 513] = ovm[B + 1]
        cm[0::2, 514] = b_out[A] * 0.25
        cm[1::2, 514] = b_out[B] * 0.25

        in_maps.append({"xin": xin, "lin": lin, "wm": wm, "bv": bv, "cm": cm})

    return in_maps, idxs, counts


def _postprocess(results, idxs, counts):
    pred = np.zeros(N, np.float32)
    for w in range(NW):
        c, s = divmod(w, 2)
        vals = results[c]["out"][s::2].reshape(NPAD)[: counts[w]]
        pred[idxs[w]] += vals
    return pred


def kernel(x, W_in, b_in, W_h, b_h, W_out, b_out):
    from concourse.bass_utils import run_bass_kernel_spmd

    in_maps, idxs, counts = _prepare(x, W_in, b_in, W_h, b_h, W_out, b_out)
    nc = _get_nc()
    res = run_bass_kernel_spmd(nc, in_maps, core_ids=list(range(NCORES)))
    return _postprocess(res.results, idxs, counts)


# revision 41
# speedup vs baseline: 1.2231x; 1.0016x over previous
"""FBPinn (16-window 1D PINN ensemble) forward pass on 8 Trainium2 NeuronCores.

Strategy (MoE-style routing, expert-parallel over windows):
  - Each of the 100k points lies strictly inside only 1-2 of the 16
    overlapping subdomains, so we route: gather the points of each window
    on the host, run each window's small MLP only on its own points, and
    scatter-add the windowed outputs back.
  - Core c owns windows (2c, 2c+1). The two windows are packed into the
    128-partition dimension (64 neurons each) so every matmul runs with
    K=M=128 via block-diagonal weights, and every tanh runs on all 128
    ACT lanes.
  - Layer 1 (per-point affine) is folded into a K=2 outer-product matmul
    with host-folded scales a = W_in/std, c = b_in - W_in*mean/std.
  - Window functions sigmoid((x-l)/s)*sigmoid(-(x-r)/s) are computed on
    device as (1+tanh(x-l))*(1+tanh(r-x))/4 (same table set as the MLP
    tanh), with the 1/4 folded into W_out/b_out on the host.
"""

import numpy as np

# Problem constants (mirrors reference.py static config)
NW = 16
D0, D1 = 0.0, 100.0
OVERLAP = 0.25
SIGMA = 0.5
NEURONS = 64
N = 100_000

NCORES = 8
NPAD = 8192          # per-window padded point count (max real count is 7930)
F = 1024             # chunk width (points) per tanh activation
NCHUNK = NPAD // F
NBLK = NPAD // 512   # 512-wide output blocks per window

_state: dict = {}


def _geometry():
    width = (D1 - D0) / NW
    i = np.arange(NW)
    lo = np.where(i == 0, D0, D0 + (i - OVERLAP / 2) * width)
    hi = np.where(i == NW - 1, D1, D0 + (i + 1 + OVERLAP / 2) * width)
    means = (lo + hi) / 2
    std = (hi - lo) / 2
    ovm = np.empty(NW + 1)
    ovm[0] = lo[0]
    ovm[NW] = hi[-1]
    ovm[1:NW] = (hi[:-1] + lo[1:]) / 2
    f32 = lambda a: np.asarray(a, np.float32)
    return f32(lo), f32(hi), f32(means), f32(std), f32(ovm)


def _build_nc():
    import concourse.bass as bass  # noqa: F401
    import concourse.tile as tile
    from concourse import bacc, mybir

    f32 = mybir.dt.float32
    f32r = mybir.dt.float32r  # 1-pass reduced-precision fp32 matmul
    AF = mybir.ActivationFunctionType
    ALU = mybir.AluOpType

    nc = bacc.Bacc("TRN2", target_bir_lowering=False, debug=False)

    # batched inputs: xin = xpair (normalized), wm = [lh0 | lh1 | lout],
    # cm = [x32 | wb], bv = [b_in | b_h0 | b_h1 | W_in]
    d_xin = nc.dram_tensor("xin", [2, NPAD], f32r, kind="ExternalInput")
    d_lin = nc.dram_tensor("lin", [2, 128], f32r, kind="ExternalInput")
    d_wm = nc.dram_tensor("wm", [128, 258], f32r, kind="ExternalInput")
    d_bv = nc.dram_tensor("bv", [128, 4], f32, kind="ExternalInput")
    d_cm = nc.dram_tensor("cm", [32, 515], f32, kind="ExternalInput")
    d_out = nc.dram_tensor("out", [32, 512], f32, kind="ExternalOutput")

    with tile.TileContext(nc) as tc:
        with (
            tc.tile_pool(name="consts", bufs=1) as cp,
            tc.tile_pool(name="hp", bufs=4) as hp,
            tc.tile_pool(name="pp", bufs=3, space="PSUM") as pp,
            tc.tile_pool(name="pop", bufs=2, space="PSUM") as pop,
            tc.tile_pool(name="mp", bufs=1) as mp,
        ):
            xpr = cp.tile([2, NPAD + 128], f32r, tag="xpr")
            nc.sync.dma_start(xpr[:, NPAD:], d_lin[:])
            nc.sync.dma_start(xpr[:, 0:NPAD], d_xin[:])
            lin = xpr[:, NPAD:]
            cm = cp.tile([32, 515], f32, tag="cm")
            nc.gpsimd.dma_start(cm[:], d_cm[:])
            wm = cp.tile([128, 258], f32r, tag="wm")
            nc.scalar.dma_start(wm[:], d_wm[:])
            bv = cp.tile([128, 4], f32, tag="bv")
            nc.gpsimd.dma_start(bv[:], d_bv[:])

            lh0 = wm[:, 0:128]
            lh1 = wm[:, 128:256]
            lout = wm[:, 256:258]
            x32 = cm[:, 0:512]
            wb = cm[:, 512:515]

            oacc2 = mp.tile([2, NPAD], f32, tag="oacc2")
            oacc = mp.tile([32, 512], f32, tag="oacc")
            h3big = mp.tile([128, NPAD], f32r, tag="h3big")

            # PE warm-up burst: dummy matmuls during the DMA-load head keep
            # the PE HAM activity window busy so real matmuls start at 2.4GHz
            # instead of the throttled 1.2GHz. Reuses the po psum slots.
            dum = mp.tile([2, 514], mybir.dt.bfloat16, tag="dum")
            nc.vector.memset(dum[:], 0.0)
            for w in range(4):
                pw = pop.tile([2, 512], f32, tag="po", name=f"warm_{w}")
                nc.tensor.matmul(pw[:], dum[:, 512:514], dum[:, 0:512],
                                 start=True, stop=True)

            # Window weights early (ACT fills while PE warms up):
            # 4*win = (1+tanh(x-ovm_w)) * (1+tanh(ovm_{w+1}-x))
            wtL = mp.tile([32, 512], f32, tag="wtL")
            nc.scalar.activation(wtL[:], x32, AF.Tanh, bias=wb[:, 0:1], scale=1.0)
            wtR = mp.tile([32, 512], f32, tag="wtR")
            nc.scalar.activation(wtR[:], x32, AF.Tanh, bias=wb[:, 1:2], scale=-1.0)
            # win4 precomputed up front; only `fin` remains in the tail
            tp = mp.tile([32, 512], f32, tag="tp")
            nc.vector.tensor_scalar_add(tp[:], wtL[:], 1.0)
            win4 = mp.tile([32, 512], f32, tag="win4")
            nc.vector.scalar_tensor_tensor(
                win4[:], wtR[:], 1.0, tp[:], op0=ALU.add, op1=ALU.mult
            )

            def emit_out(j):
                for s in range(F // 512):
                    n = j * (F // 512) + s
                    pout = pop.tile([2, 512], f32, tag="po", name=f"po_{n}")
                    nc.tensor.matmul(
                        pout[:], lout, h3big[:, n * 512 : (n + 1) * 512],
                        start=True, stop=True,
                    )
                    nc.vector.tensor_copy(oacc2[:, n * 512 : (n + 1) * 512], pout[:])
                    # reshuffle into [32, 512] combine layout as soon as ready
                    nc.sync.dma_start(
                        oacc[2 * n : 2 * n + 2, :], oacc2[:, n * 512 : (n + 1) * 512]
                    )

            # ---- skewed software pipeline (wavefront over chunks) ----
            h1s, h2s = {}, {}

            def stage1(j):
                xsl = slice(j * F, (j + 1) * F)
                p1 = pp.tile([128, F], f32, tag="ps", name=f"p1_{j}")
                for s in range(F // 512):
                    sl = slice(s * 512, (s + 1) * 512)
                    xs = slice(j * F + s * 512, j * F + (s + 1) * 512)
                    nc.tensor.matmul(p1[:, sl], lin, xpr[:, xs], start=True, stop=True)
                h1 = hp.tile([128, F], f32r, tag="h1", name=f"h1_{j}")
                nc.scalar.activation(h1[:], p1[:], AF.Tanh, bias=bv[:, 0:1])
                h1s[j] = h1

            def stage2(j):
                p2 = pp.tile([128, F], f32, tag="ps", name=f"p2_{j}")
                for s in range(F // 512):
                    sl = slice(s * 512, (s + 1) * 512)
                    nc.tensor.matmul(p2[:, sl], lh0, h1s[j][:, sl], start=True, stop=True)
                h2 = hp.tile([128, F], f32r, tag="h2", name=f"h2_{j}")
                nc.scalar.activation(h2[:], p2[:], AF.Tanh, bias=bv[:, 1:2])
                h2s[j] = h2

            def stage3(j):
                p3 = pp.tile([128, F], f32, tag="ps", name=f"p3_{j}")
                for s in range(F // 512):
                    sl = slice(s * 512, (s + 1) * 512)
                    nc.tensor.matmul(p3[:, sl], lh1, h2s[j][:, sl], start=True, stop=True)
                nc.scalar.activation(
                    h3big[:, j * F : (j + 1) * F], p3[:], AF.Tanh, bias=bv[:, 2:3]
                )
                emit_out(j)

            for t in range(NCHUNK + 2):
                if t < NCHUNK:
                    stage1(t)
                if 1 <= t < NCHUNK + 1:
                    stage2(t - 1)
                if t >= 2:
                    stage3(t - 2)

            fin = mp.tile([32, 512], f32, tag="fin")
            nc.vector.scalar_tensor_tensor(
                fin[:], oacc[:], wb[:, 2:3], win4[:], op0=ALU.add, op1=ALU.mult
            )
            nc.sync.dma_start(d_out[:], fin[:])

    nc.compile()
    return nc


def _get_nc():
    if "nc" not in _state:
        _state["nc"] = _build_nc()
    return _state["nc"]


def _prepare(x, W_in, b_in, W_h, b_h, W_out, b_out):
    x = np.asarray(x, np.float32)
    W_in = np.asarray(W_in, np.float32)
    b_in = np.asarray(b_in, np.float32)
    W_h = np.asarray(W_h, np.float32)
    b_h = np.asarray(b_h, np.float32)
    W_out = np.asarray(W_out, np.float32)
    b_out = np.asarray(b_out, np.float32)

    lo, hi, means, std, ovm = _geometry()

    # ---- host routing: gather each window's points ----
    idxs, counts = [], []
    for w in range(NW):
        idx = np.nonzero((lo[w] < x) & (x < hi[w]))[0]
        assert len(idx) <= NPAD, f"window {w} has {len(idx)} points > NPAD={NPAD}"
        idxs.append(idx)
        counts.append(len(idx))

    in_maps = []
    for c in range(NCORES):
        A, B = 2 * c, 2 * c + 1
        xA = np.full(NPAD, means[A], np.float32)
        xA[: counts[A]] = x[idxs[A]]
        xB = np.full(NPAD, means[B], np.float32)
        xB[: counts[B]] = x[idxs[B]]
        # normalized per-window inputs (matches reference's xn exactly)
        xin = np.stack([(xA - means[A]) / std[A], (xB - means[B]) / std[B]])

        lin = np.zeros((2, 128), np.float32)
        lin[0, :64] = W_in[A]
        lin[1, 64:] = W_in[B]

        bv = np.empty((128, 4), np.float32)
        bv[:64, 0] = b_in[A]
        bv[64:, 0] = b_in[B]
        bv[:64, 1] = b_h[0, A]
        bv[64:, 1] = b_h[0, B]
        bv[:64, 2] = b_h[1, A]
        bv[64:, 2] = b_h[1, B]
        bv[:64, 3] = W_in[A]
        bv[64:, 3] = W_in[B]

        # wm = [lh0 | lh1 | lout]
        wm = np.zeros((128, 258), np.float32)
        wm[:64, 0:64] = W_h[0, A]
        wm[64:, 64:128] = W_h[0, B]
        wm[:64, 128:192] = W_h[1, A]
        wm[64:, 192:256] = W_h[1, B]
        wm[:64, 256] = W_out[A] * 0.25
        wm[64:, 257] = W_out[B] * 0.25

        # cm = [x32 | wb]: row 2n = window-A 512-block n, row 2n+1 = B block n
        # wb: col0 = left-edge tanh bias (-ovm_w), col1 = right-edge tanh
        # bias (+ovm_{w+1}), col2 = b_out/4
        cm = np.empty((32, 515), np.float32)
        cm[0::2, :512] = xA.reshape(NBLK, 512)
        cm[1::2, :512] = xB.reshape(NBLK, 512)
        cm[0::2, 512] = -ovm[A]
        cm[1::2, 512] = -ovm[B]
        cm[0::2, 513] = ovm[A + 1]
        cm[1::2, 513] = ovm[B + 1]
        cm[0::2, 514] = b_out[A] * 0.25
        cm[1::2, 514] = b_out[B] * 0.25

        in_maps.append({"xin": xin, "lin": lin, "wm": wm, "bv": bv, "cm": cm})

    return in_maps, idxs, counts


def _postprocess(results, idxs, counts):
    pred = np.zeros(N, np.float32)
    for w in range(NW):
        c, s = divmod(w, 2)
        vals = results[c]["out"][s::2].reshape(NPAD)[: counts[w]]
        pred[idxs[w]] += vals
    return pred


def kernel(x, W_in, b_in, W_h, b_h, W_out, b_out):
    from concourse.bass_utils import run_bass_kernel_spmd

    in_maps, idxs, counts = _prepare(x, W_in, b_in, W_h, b_h, W_out, b_out)
    nc = _get_nc()
    res = run_bass_kernel_spmd(nc, in_maps, core_ids=list(range(NCORES)))
    return _postprocess(res.results, idxs, counts)


# revision 43
# speedup vs baseline: 1.2890x; 1.0538x over previous
"""FBPinn (16-window 1D PINN ensemble) forward pass on 8 Trainium2 NeuronCores.

Strategy (MoE-style routing, expert-parallel over windows):
  - Each of the 100k points lies strictly inside only 1-2 of the 16
    overlapping subdomains, so we route: gather the points of each window
    on the host, run each window's small MLP only on its own points, and
    scatter-add the windowed outputs back.
  - Core c owns windows (2c, 2c+1). The two windows are packed into the
    128-partition dimension (64 neurons each) so every matmul runs with
    K=M=128 via block-diagonal weights, and every tanh runs on all 128
    ACT lanes.
  - Layer 1 (per-point affine) is folded into a K=2 outer-product matmul
    with host-folded scales a = W_in/std, c = b_in - W_in*mean/std.
  - Window functions sigmoid((x-l)/s)*sigmoid(-(x-r)/s) are computed on
    device as (1+tanh(x-l))*(1+tanh(r-x))/4 (same table set as the MLP
    tanh), with the 1/4 folded into W_out/b_out on the host.
"""

import numpy as np

# Problem constants (mirrors reference.py static config)
NW = 16
D0, D1 = 0.0, 100.0
OVERLAP = 0.25
SIGMA = 0.5
NEURONS = 64
N = 100_000

NCORES = 8
NPAD = 8192          # per-window padded point count (max real count is 7930)
F = 1024             # chunk width (points) per tanh activation
NCHUNK = NPAD // F
NBLK = NPAD // 512   # 512-wide output blocks per window

_state: dict = {}


def _geometry():
    width = (D1 - D0) / NW
    i = np.arange(NW)
    lo = np.where(i == 0, D0, D0 + (i - OVERLAP / 2) * width)
    hi = np.where(i == NW - 1, D1, D0 + (i + 1 + OVERLAP / 2) * width)
    means = (lo + hi) / 2
    std = (hi - lo) / 2
    ovm = np.empty(NW + 1)
    ovm[0] = lo[0]
    ovm[NW] = hi[-1]
    ovm[1:NW] = (hi[:-1] + lo[1:]) / 2
    f32 = lambda a: np.asarray(a, np.float32)
    return f32(lo), f32(hi), f32(means), f32(std), f32(ovm)


def _build_nc():
    import concourse.bass as bass  # noqa: F401
    import concourse.tile as tile
    from concourse import bacc, mybir

    f32 = mybir.dt.float32
    f32r = mybir.dt.float32r  # 1-pass reduced-precision fp32 matmul
    AF = mybir.ActivationFunctionType
    ALU = mybir.AluOpType

    nc = bacc.Bacc("TRN2", target_bir_lowering=False, debug=False)

    # batched inputs: xin = xpair (normalized), wm = [lh0 | lh1 | lout],
    # cm = [x32 | wb], bv = [b_in | b_h0 | b_h1 | W_in]
    d_xin = nc.dram_tensor("xin", [2, NPAD], f32r, kind="ExternalInput")
    d_lin = nc.dram_tensor("lin", [2, 128], f32r, kind="ExternalInput")
    d_wm = nc.dram_tensor("wm", [128, 258], f32r, kind="ExternalInput")
    d_bv = nc.dram_tensor("bv", [128, 4], f32, kind="ExternalInput")
    d_cm = nc.dram_tensor("cm", [32, 515], f32, kind="ExternalInput")
    d_bo2 = nc.dram_tensor("bo2", [2, 1], f32, kind="ExternalInput")
    d_out = nc.dram_tensor("out", [2, NPAD], f32, kind="ExternalOutput")

    with tile.TileContext(nc) as tc:
        with (
            tc.tile_pool(name="consts", bufs=1) as cp,
            tc.tile_pool(name="hp", bufs=4) as hp,
            tc.tile_pool(name="pp", bufs=3, space="PSUM") as pp,
            tc.tile_pool(name="pop", bufs=2, space="PSUM") as pop,
            tc.tile_pool(name="mp", bufs=1) as mp,
        ):
            xpr = cp.tile([2, NPAD + 128], f32r, tag="xpr")
            nc.sync.dma_start(xpr[:, NPAD:], d_lin[:])
            nc.sync.dma_start(xpr[:, 0:NPAD], d_xin[:])
            lin = xpr[:, NPAD:]
            cm = cp.tile([32, 515], f32, tag="cm")
            nc.gpsimd.dma_start(cm[:], d_cm[:])
            wm = cp.tile([128, 258], f32r, tag="wm")
            nc.scalar.dma_start(wm[:], d_wm[:])
            bv = cp.tile([128, 4], f32, tag="bv")
            nc.gpsimd.dma_start(bv[:], d_bv[:])
            bo2 = cp.tile([2, 1], f32, tag="bo2")
            nc.gpsimd.dma_start(bo2[:], d_bo2[:])

            lh0 = wm[:, 0:128]
            lh1 = wm[:, 128:256]
            lout = wm[:, 256:258]
            x32 = cm[:, 0:512]
            wb = cm[:, 512:515]

            win2 = mp.tile([2, NPAD], f32, tag="win2")
            fin2 = mp.tile([2, NPAD], f32, tag="fin2")
            h3big = mp.tile([128, NPAD], f32r, tag="h3big")

            # PE warm-up burst: dummy matmuls during the DMA-load head keep
            # the PE HAM activity window busy so real matmuls start at 2.4GHz
            # instead of the throttled 1.2GHz. Reuses the po psum slots.
            dum = mp.tile([2, 514], mybir.dt.bfloat16, tag="dum")
            nc.vector.memset(dum[:], 0.0)
            for w in range(4):
                pw = pop.tile([2, 512], f32, tag="po", name=f"warm_{w}")
                nc.tensor.matmul(pw[:], dum[:, 512:514], dum[:, 0:512],
                                 start=True, stop=True)

            # Window weights early (ACT fills while PE warms up):
            # 4*win = (1+tanh(x-ovm_w)) * (1+tanh(ovm_{w+1}-x))
            wtL = mp.tile([32, 512], f32, tag="wtL")
            nc.scalar.activation(wtL[:], x32, AF.Tanh, bias=wb[:, 0:1], scale=1.0)
            wtR = mp.tile([32, 512], f32, tag="wtR")
            nc.scalar.activation(wtR[:], x32, AF.Tanh, bias=wb[:, 1:2], scale=-1.0)
            # win4 precomputed up front; only `fin` remains in the tail
            tp = mp.tile([32, 512], f32, tag="tp")
            nc.vector.tensor_scalar_add(tp[:], wtL[:], 1.0)
            win4 = mp.tile([32, 512], f32, tag="win4")
            nc.vector.scalar_tensor_tensor(
                win4[:], wtR[:], 1.0, tp[:], op0=ALU.add, op1=ALU.mult
            )
            # reshuffle win4 into [2, NPAD] early (off the critical tail)
            for n in range(NBLK):
                nc.sync.dma_start(
                    win2[:, n * 512 : (n + 1) * 512], win4[2 * n : 2 * n + 2, :]
                )

            def emit_out(j):
                for s in range(F // 512):
                    n = j * (F // 512) + s
                    sl = slice(n * 512, (n + 1) * 512)
                    pout = pop.tile([2, 512], f32, tag="po", name=f"po_{n}")
                    nc.tensor.matmul(
                        pout[:], lout, h3big[:, sl], start=True, stop=True,
                    )
                    # fused combine: fin = (out + b_out/4) * 4*win
                    nc.vector.scalar_tensor_tensor(
                        fin2[:, sl], pout[:], bo2[:, 0:1], win2[:, sl],
                        op0=ALU.add, op1=ALU.mult,
                    )
                    nc.sync.dma_start(d_out[:, sl], fin2[:, sl])

            # ---- skewed software pipeline (wavefront over chunks) ----
            h1s, h2s = {}, {}

            def stage1(j):
                xsl = slice(j * F, (j + 1) * F)
                p1 = pp.tile([128, F], f32, tag="ps", name=f"p1_{j}")
                for s in range(F // 512):
                    sl = slice(s * 512, (s + 1) * 512)
                    xs = slice(j * F + s * 512, j * F + (s + 1) * 512)
                    nc.tensor.matmul(p1[:, sl], lin, xpr[:, xs], start=True, stop=True)
                h1 = hp.tile([128, F], f32r, tag="h1", name=f"h1_{j}")
                nc.scalar.activation(h1[:], p1[:], AF.Tanh, bias=bv[:, 0:1])
                h1s[j] = h1

            def stage2(j):
                p2 = pp.tile([128, F], f32, tag="ps", name=f"p2_{j}")
                for s in range(F // 512):
                    sl = slice(s * 512, (s + 1) * 512)
                    nc.tensor.matmul(p2[:, sl], lh0, h1s[j][:, sl], start=True, stop=True)
                h2 = hp.tile([128, F], f32r, tag="h2", name=f"h2_{j}")
                nc.scalar.activation(h2[:], p2[:], AF.Tanh, bias=bv[:, 1:2])
                h2s[j] = h2

            def stage3(j):
                p3 = pp.tile([128, F], f32, tag="ps", name=f"p3_{j}")
                for s in range(F // 512):
                    sl = slice(s * 512, (s + 1) * 512)
                    nc.tensor.matmul(p3[:, sl], lh1, h2s[j][:, sl], start=True, stop=True)
                nc.scalar.activation(
                    h3big[:, j * F : (j + 1) * F], p3[:], AF.Tanh, bias=bv[:, 2:3]
                )
                emit_out(j)

            for t in range(NCHUNK + 2):
                if t < NCHUNK:
                    stage1(t)
                if 1 <= t < NCHUNK + 1:
                    stage2(t - 1)
                if t >= 2:
                    stage3(t - 2)


    nc.compile()
    return nc


def _get_nc():
    if "nc" not in _state:
        _state["nc"] = _build_nc()
    return _state["nc"]


def _prepare(x, W_in, b_in, W_h, b_h, W_out, b_out):
    x = np.asarray(x, np.float32)
    W_in = np.asarray(W_in, np.float32)
    b_in = np.asarray(b_in, np.float32)
    W_h = np.asarray(W_h, np.float32)
    b_h = np.asarray(b_h, np.float32)
    W_out = np.asarray(W_out, np.float32)
    b_out = np.asarray(b_out, np.float32)

    lo, hi, means, std, ovm = _geometry()

    # ---- host routing: gather each window's points ----
    idxs, counts = [], []
    for w in range(NW):
        idx = np.nonzero((lo[w] < x) & (x < hi[w]))[0]
        assert len(idx) <= NPAD, f"window {w} has {len(idx)} points > NPAD={NPAD}"
        idxs.append(idx)
        counts.append(len(idx))

    in_maps = []
    for c in range(NCORES):
        A, B = 2 * c, 2 * c + 1
        xA = np.full(NPAD, means[A], np.float32)
        xA[: counts[A]] = x[idxs[A]]
        xB = np.full(NPAD, means[B], np.float32)
        xB[: counts[B]] = x[idxs[B]]
        # normalized per-window inputs (matches reference's xn exactly)
        xin = np.stack([(xA - means[A]) / std[A], (xB - means[B]) / std[B]])

        lin = np.zeros((2, 128), np.float32)
        lin[0, :64] = W_in[A]
        lin[1, 64:] = W_in[B]

        bv = np.empty((128, 4), np.float32)
        bv[:64, 0] = b_in[A]
        bv[64:, 0] = b_in[B]
        bv[:64, 1] = b_h[0, A]
        bv[64:, 1] = b_h[0, B]
        bv[:64, 2] = b_h[1, A]
        bv[64:, 2] = b_h[1, B]
        bv[:64, 3] = W_in[A]
        bv[64:, 3] = W_in[B]

        # wm = [lh0 | lh1 | lout]
        wm = np.zeros((128, 258), np.float32)
        wm[:64, 0:64] = W_h[0, A]
        wm[64:, 64:128] = W_h[0, B]
        wm[:64, 128:192] = W_h[1, A]
        wm[64:, 192:256] = W_h[1, B]
        wm[:64, 256] = W_out[A] * 0.25
        wm[64:, 257] = W_out[B] * 0.25

        # cm = [x32 | wb]: row 2n = window-A 512-block n, row 2n+1 = B block n
        # wb: col0 = left-edge tanh bias (-ovm_w), col1 = right-edge tanh
        # bias (+ovm_{w+1}), col2 = b_out/4
        cm = np.empty((32, 515), np.float32)
        cm[0::2, :512] = xA.reshape(NBLK, 512)
        cm[1::2, :512] = xB.reshape(NBLK, 512)
        cm[0::2, 512] = -ovm[A]
        cm[1::2, 512] = -ovm[B]
        cm[0::2, 513] = ovm[A + 1]
        cm[1::2, 513] = ovm[B + 1]
        cm[0::2, 514] = b_out[A] * 0.25
        cm[1::2, 514] = b_out[B] * 0.25

        bo2 = np.array([[b_out[A] * 0.25], [b_out[B] * 0.25]], np.float32)

        in_maps.append(
            {"xin": xin, "lin": lin, "wm": wm, "bv": bv, "cm": cm, "bo2": bo2}
        )

    return in_maps, idxs, counts


def _postprocess(results, idxs, counts):
    pred = np.zeros(N, np.float32)
    for w in range(NW):
        c, s = divmod(w, 2)
        vals = results[c]["out"][s, : counts[w]]
        pred[idxs[w]] += vals
    return pred


def kernel(x, W_in, b_in, W_h, b_h, W_out, b_out):
    from concourse.bass_utils import run_bass_kernel_spmd

    in_maps, idxs, counts = _prepare(x, W_in, b_in, W_h, b_h, W_out, b_out)
    nc = _get_nc()
    res = run_bass_kernel_spmd(nc, in_maps, core_ids=list(range(NCORES)))
    return _postprocess(res.results, idxs, counts)


# revision 44
# speedup vs baseline: 1.2950x; 1.0047x over previous
"""FBPinn (16-window 1D PINN ensemble) forward pass on 8 Trainium2 NeuronCores.

Strategy (MoE-style routing, expert-parallel over windows):
  - Each of the 100k points lies strictly inside only 1-2 of the 16
    overlapping subdomains, so we route: gather the points of each window
    on the host, run each window's small MLP only on its own points, and
    scatter-add the windowed outputs back.
  - Core c owns windows (2c, 2c+1). The two windows are packed into the
    128-partition dimension (64 neurons each) so every matmul runs with
    K=M=128 via block-diagonal weights, and every tanh runs on all 128
    ACT lanes.
  - Layer 1 (per-point affine) is folded into a K=2 outer-product matmul
    with host-folded scales a = W_in/std, c = b_in - W_in*mean/std.
  - Window functions sigmoid((x-l)/s)*sigmoid(-(x-r)/s) are computed on
    device as (1+tanh(x-l))*(1+tanh(r-x))/4 (same table set as the MLP
    tanh), with the 1/4 folded into W_out/b_out on the host.
"""

import numpy as np

# Problem constants (mirrors reference.py static config)
NW = 16
D0, D1 = 0.0, 100.0
OVERLAP = 0.25
SIGMA = 0.5
NEURONS = 64
N = 100_000

NCORES = 8
NPAD = 8192          # per-window padded point count (max real count is 7930)
F = 1024             # chunk width (points) per tanh activation
NCHUNK = NPAD // F
NBLK = NPAD // 512   # 512-wide output blocks per window

_state: dict = {}


def _geometry():
    width = (D1 - D0) / NW
    i = np.arange(NW)
    lo = np.where(i == 0, D0, D0 + (i - OVERLAP / 2) * width)
    hi = np.where(i == NW - 1, D1, D0 + (i + 1 + OVERLAP / 2) * width)
    means = (lo + hi) / 2
    std = (hi - lo) / 2
    ovm = np.empty(NW + 1)
    ovm[0] = lo[0]
    ovm[NW] = hi[-1]
    ovm[1:NW] = (hi[:-1] + lo[1:]) / 2
    f32 = lambda a: np.asarray(a, np.float32)
    return f32(lo), f32(hi), f32(means), f32(std), f32(ovm)


def _build_nc():
    import concourse.bass as bass  # noqa: F401
    import concourse.tile as tile
    from concourse import bacc, mybir

    f32 = mybir.dt.float32
    f32r = mybir.dt.float32r  # 1-pass reduced-precision fp32 matmul
    AF = mybir.ActivationFunctionType
    ALU = mybir.AluOpType

    nc = bacc.Bacc("TRN2", target_bir_lowering=False, debug=False)

    # batched inputs: xin = xpair (normalized), wm = [lh0 | lh1 | lout],
    # cm = [x32 | wb], bv = [b_in | b_h0 | b_h1 | W_in]
    d_xin = nc.dram_tensor("xin", [2, NPAD], f32r, kind="ExternalInput")
    d_lin = nc.dram_tensor("lin", [2, 128], f32r, kind="ExternalInput")
    d_wm = nc.dram_tensor("wm", [128, 258], f32r, kind="ExternalInput")
    d_bv = nc.dram_tensor("bv", [128, 4], f32, kind="ExternalInput")
    d_win2 = nc.dram_tensor("win2", [2, NPAD], f32, kind="ExternalInput")
    d_bo2 = nc.dram_tensor("bo2", [2, 1], f32, kind="ExternalInput")
    d_out = nc.dram_tensor("out", [2, NPAD], f32, kind="ExternalOutput")

    with tile.TileContext(nc) as tc:
        with (
            tc.tile_pool(name="consts", bufs=1) as cp,
            tc.tile_pool(name="hp", bufs=4) as hp,
            tc.tile_pool(name="pp", bufs=3, space="PSUM") as pp,
            tc.tile_pool(name="pop", bufs=2, space="PSUM") as pop,
            tc.tile_pool(name="mp", bufs=1) as mp,
        ):
            xpr = cp.tile([2, NPAD + 128], f32r, tag="xpr")
            nc.sync.dma_start(xpr[:, NPAD:], d_lin[:])
            nc.sync.dma_start(xpr[:, 0:NPAD], d_xin[:])
            lin = xpr[:, NPAD:]
            wm = cp.tile([128, 258], f32r, tag="wm")
            nc.scalar.dma_start(wm[:], d_wm[:])
            bv = cp.tile([128, 4], f32, tag="bv")
            nc.sync.dma_start(bv[:], d_bv[:])
            bo2 = cp.tile([2, 1], f32, tag="bo2")
            nc.sync.dma_start(bo2[:], d_bo2[:])
            win2 = cp.tile([2, NPAD], f32, tag="win2")
            nc.sync.dma_start(win2[:], d_win2[:])

            lh0 = wm[:, 0:128]
            lh1 = wm[:, 128:256]
            lout = wm[:, 256:258]

            fin2 = mp.tile([2, NPAD], f32, tag="fin2")
            h3big = mp.tile([128, NPAD], f32r, tag="h3big")

            # PE warm-up burst: dummy matmuls during the DMA-load head keep
            # the PE HAM activity window busy so real matmuls start at 2.4GHz
            # instead of the throttled 1.2GHz. Reuses the po psum slots.
            dum = mp.tile([2, 514], mybir.dt.bfloat16, tag="dum")
            nc.vector.memset(dum[:], 0.0)
            for w in range(4):
                pw = pop.tile([2, 512], f32, tag="po", name=f"warm_{w}")
                nc.tensor.matmul(pw[:], dum[:, 512:514], dum[:, 0:512],
                                 start=True, stop=True)


            def emit_out(j):
                for s in range(F // 512):
                    n = j * (F // 512) + s
                    sl = slice(n * 512, (n + 1) * 512)
                    pout = pop.tile([2, 512], f32, tag="po", name=f"po_{n}")
                    nc.tensor.matmul(
                        pout[:], lout, h3big[:, sl], start=True, stop=True,
                    )
                    # fused combine: fin = (out + b_out/4) * 4*win
                    nc.vector.scalar_tensor_tensor(
                        fin2[:, sl], pout[:], bo2[:, 0:1], win2[:, sl],
                        op0=ALU.add, op1=ALU.mult,
                    )
                    nc.sync.dma_start(d_out[:, sl], fin2[:, sl])

            # ---- skewed software pipeline (wavefront over chunks) ----
            h1s, h2s = {}, {}

            def stage1(j):
                xsl = slice(j * F, (j + 1) * F)
                p1 = pp.tile([128, F], f32, tag="ps", name=f"p1_{j}")
                for s in range(F // 512):
                    sl = slice(s * 512, (s + 1) * 512)
                    xs = slice(j * F + s * 512, j * F + (s + 1) * 512)
                    nc.tensor.matmul(p1[:, sl], lin, xpr[:, xs], start=True, stop=True)
                h1 = hp.tile([128, F], f32r, tag="h1", name=f"h1_{j}")
                nc.scalar.activation(h1[:], p1[:], AF.Tanh, bias=bv[:, 0:1])
                h1s[j] = h1

            def stage2(j):
                p2 = pp.tile([128, F], f32, tag="ps", name=f"p2_{j}")
                for s in range(F // 512):
                    sl = slice(s * 512, (s + 1) * 512)
                    nc.tensor.matmul(p2[:, sl], lh0, h1s[j][:, sl], start=True, stop=True)
                h2 = hp.tile([128, F], f32r, tag="h2", name=f"h2_{j}")
                nc.scalar.activation(h2[:], p2[:], AF.Tanh, bias=bv[:, 1:2])
                h2s[j] = h2

            def stage3(j):
                p3 = pp.tile([128, F], f32, tag="ps", name=f"p3_{j}")
                for s in range(F // 512):
                    sl = slice(s * 512, (s + 1) * 512)
                    nc.tensor.matmul(p3[:, sl], lh1, h2s[j][:, sl], start=True, stop=True)
                nc.scalar.activation(
                    h3big[:, j * F : (j + 1) * F], p3[:], AF.Tanh, bias=bv[:, 2:3]
                )
                emit_out(j)

            for t in range(NCHUNK + 2):
                if t < NCHUNK:
                    stage1(t)
                if 1 <= t < NCHUNK + 1:
                    stage2(t - 1)
                if t >= 2:
                    stage3(t - 2)


    nc.compile()
    return nc


def _get_nc():
    if "nc" not in _state:
        _state["nc"] = _build_nc()
    return _state["nc"]


def _prepare(x, W_in, b_in, W_h, b_h, W_out, b_out):
    x = np.asarray(x, np.float32)
    W_in = np.asarray(W_in, np.float32)
    b_in = np.asarray(b_in, np.float32)
    W_h = np.asarray(W_h, np.float32)
    b_h = np.asarray(b_h, np.float32)
    W_out = np.asarray(W_out, np.float32)
    b_out = np.asarray(b_out, np.float32)

    lo, hi, means, std, ovm = _geometry()

    # ---- host routing: gather each window's points ----
    idxs, counts = [], []
    for w in range(NW):
        idx = np.nonzero((lo[w] < x) & (x < hi[w]))[0]
        assert len(idx) <= NPAD, f"window {w} has {len(idx)} points > NPAD={NPAD}"
        idxs.append(idx)
        counts.append(len(idx))

    in_maps = []
    for c in range(NCORES):
        A, B = 2 * c, 2 * c + 1
        xA = np.full(NPAD, means[A], np.float32)
        xA[: counts[A]] = x[idxs[A]]
        xB = np.full(NPAD, means[B], np.float32)
        xB[: counts[B]] = x[idxs[B]]
        # normalized per-window inputs (matches reference's xn exactly)
        xin = np.stack([(xA - means[A]) / std[A], (xB - means[B]) / std[B]])

        lin = np.zeros((2, 128), np.float32)
        lin[0, :64] = W_in[A]
        lin[1, 64:] = W_in[B]

        bv = np.empty((128, 4), np.float32)
        bv[:64, 0] = b_in[A]
        bv[64:, 0] = b_in[B]
        bv[:64, 1] = b_h[0, A]
        bv[64:, 1] = b_h[0, B]
        bv[:64, 2] = b_h[1, A]
        bv[64:, 2] = b_h[1, B]
        bv[:64, 3] = W_in[A]
        bv[64:, 3] = W_in[B]

        # wm = [lh0 | lh1 | lout]
        wm = np.zeros((128, 258), np.float32)
        wm[:64, 0:64] = W_h[0, A]
        wm[64:, 64:128] = W_h[0, B]
        wm[:64, 128:192] = W_h[1, A]
        wm[64:, 192:256] = W_h[1, B]
        wm[:64, 256] = W_out[A] * 0.25
        wm[64:, 257] = W_out[B] * 0.25

        # window routing weights, host-side (float64 sigmoids), scaled by 4
        # to match the b_out/4, W_out/4 folding: fin = (out + b/4) * (4*win)
        def win4_of(xw, w):
            z1 = 1.0 / (1.0 + np.exp(-(xw.astype(np.float64) - ovm[w]) / SIGMA))
            z2 = 1.0 / (1.0 + np.exp((xw.astype(np.float64) - ovm[w + 1]) / SIGMA))
            return (4.0 * z1 * z2).astype(np.float32)

        win2 = np.stack([win4_of(xA, A), win4_of(xB, B)])

        bo2 = np.array([[b_out[A] * 0.25], [b_out[B] * 0.25]], np.float32)

        in_maps.append(
            {"xin": xin, "lin": lin, "wm": wm, "bv": bv, "win2": win2, "bo2": bo2}
        )

    return in_maps, idxs, counts


def _postprocess(results, idxs, counts):
    pred = np.zeros(N, np.float32)
    for w in range(NW):
        c, s = divmod(w, 2)
        vals = results[c]["out"][s, : counts[w]]
        pred[idxs[w]] += vals
    return pred


def kernel(x, W_in, b_in, W_h, b_h, W_out, b_out):
    from concourse.bass_utils import run_bass_kernel_spmd

    in_maps, idxs, counts = _prepare(x, W_in, b_in, W_h, b_h, W_out, b_out)
    nc = _get_nc()
    res = run_bass_kernel_spmd(nc, in_maps, core_ids=list(range(NCORES)))
    return _postprocess(res.results, idxs, counts)
